# revision 28
# baseline (speedup 1.0000x reference)
"""BertAttention Trainium2 kernel (8 NeuronCores, SPMD).

Sharding: core c handles batch b = c//2 and head-half hh = c%2 (8 of 16 heads).
Each core computes q/k/v projections for its 512 head-dims over its batch's
full sequence, per-head attention (no mask, scale 1/sqrt(1024)), and a partial
o-projection over its 512 context dims.  Host sums the two partials per batch.

Device layout (per core):
  xt   [8,128,2048]  f32r  hidden[b].T, d-major chunks
  wqt  [4,8,128,128] f32r  w_q rows for our heads, transposed, (pair, k) chunks
  wkt  [4,8,128,128] f32r  same for w_k
  wvt  [2,8,128,256] f32r  w_v rows transposed, (col-half, k) chunks
  wot  [4,128,1024]  f32r  w_o cols for our heads, transposed, pair chunks
  outt [8,128,2048]  f32   out_partial.T (o-major chunks)

Attention per head-pair p (heads 2p, 2p+1 local):
  QT/KT [128, 2048] = heads' q/k transposed (head on partitions 0:64 / 64:128)
  S^T tile [128k, 2, 512q]: two row-packed matmuls (K=64 at base 0 and 64)
  exp: one activation over [128, 1024] with fused 1/32 scale -> f32r
  AV: per head, lhsT = V_aug[kt][:, head, 0:65] (64 v-cols + ones col),
      rhs = P^T chunk, accumulated over 16 k-tiles -> psum [65, 512]
      row 64 = softmax denominator.
  norm: den row -> ones-matmul broadcast [64,512] -> DVE recip -> DVE mult
"""

import sys

sys.path.insert(0, "/opt/trn_rl_repo")

import numpy as np

B, S, D, H = 4, 2048, 1024, 16
HEAD = 64
NCORES = 8
P = 128
NQ = 512            # q free-tile width
KT_TILES = S // P   # 16 k tiles
QT_TILES = S // NQ  # 4 q tiles
DC = 8              # contraction chunks for projections (1024/128)
PAIRS = 4           # head pairs per core


def _round_fp32r(x: np.ndarray) -> np.ndarray:
    """RNE-round fp32 to the 11-bit-mantissa fp32r grid (low 12 bits zero)."""
    b = np.ascontiguousarray(x, np.float32).view(np.uint32).astype(np.uint64)
    lsb = (b >> 12) & 1
    b = b + 0x7FF + lsb
    b = (b & 0xFFFFF000) & 0xFFFFFFFF
    return b.astype(np.uint32).view(np.float32)


_NC_CACHE = None


def _build_nc():
    import concourse.bass as bass  # noqa: F401
    import concourse.tile as tile
    from concourse import bacc, mybir

    f32 = mybir.dt.float32
    f32r = mybir.dt.float32r
    f16 = mybir.dt.float16
    AF = mybir.ActivationFunctionType

    nc = bacc.Bacc(None)
    xt_d = nc.declare_dram_parameter("xt", [DC, P, S], f16, isOutput=False)
    wqt_d = nc.declare_dram_parameter("wqt", [PAIRS, DC, P, P], f16, isOutput=False)
    wkt_d = nc.declare_dram_parameter("wkt", [PAIRS, DC, P, P], f16, isOutput=False)
    wvt_d = nc.declare_dram_parameter("wvt", [DC, P, 512], f16, isOutput=False)
    woth_d = nc.declare_dram_parameter("woth", [PAIRS, P, D], f16, isOutput=False)
    out_d = nc.declare_dram_parameter("outt", [D // P, P, S], f32, isOutput=True)

    from contextlib import ExitStack

    with tile.TileContext(nc) as tc, ExitStack() as es:
        def pool(name, bufs, space="SBUF"):
            return es.enter_context(
                tc.tile_pool(name=name, bufs=bufs, space=space))

        xt_pool = pool("xt", 1)
        wq_pool = pool("wq", 8)
        wk_pool = pool("wk", 8)
        wv_pool = pool("wv", 8)
        qt_pool = pool("qt", 2)
        kt_pool = pool("kt", 2)
        v_pool = pool("v", 1)
        pt_pool = pool("pt", 4)
        ctx_pool = pool("ctx", 1)
        wot_pool = pool("wot", 1)
        ost_pool = pool("ost", 1)
        dn_pool = pool("dn", 1)
        rc_pool = pool("rc", 1)
        on_pool = pool("on", 1)
        pp_pool = pool("pp", 2, "PSUM")
        st_pool = pool("st", 2, "PSUM")
        av_pool = pool("av", 2, "PSUM")

        if True:
            # ones row (f32r) for the denominator broadcast matmul
            ones_f = on_pool.tile([P, P], f32, tag="onf", name="onesf")
            nc.vector.memset(ones_f[:], 1.0)
            ones_r = on_pool.tile([P, P], f32r, tag="onr", name="onesr")
            nc.vector.tensor_copy(ones_r[:], ones_f[:])

            # PE warmup during the initial DMA: keeps HAM at 8/8 so the
            # first projection matmuls run at 2.4 GHz
            wup = on_pool.tile([P, NQ], f16, tag="wup", name="wup")
            nc.vector.memset(wup[:], 0.125)
            wups = pp_pool.tile([P, NQ], f32, tag="pp", name="wups")
            for _ in range(30):
                nc.tensor.matmul(wups[:], wup[:, 0:P], wup[:],
                                 start=True, stop=True)

            # load x^T chunks
            xt = []
            for k in range(DC):
                t = xt_pool.tile([P, S], f16, tag=f"xt{k}", name=f"xt{k}")
                eng = nc.sync if k % 2 == 0 else nc.scalar
                eng.dma_start(t[:], xt_d[k])
                xt.append(t)

            # V_aug: separate tiles per head-half (heads 4h..4h+3); ones col
            # per head at offset 65h+64.  One N=512 projection pass fills both.
            v_half = {0: [None] * KT_TILES, 1: [None] * KT_TILES}
            wv_t = []

            def load_wv():
                for k in range(DC):
                    t = wv_pool.tile([P, NQ], f16, tag="wv", name="wv")
                    nc.scalar.dma_start(t[:], wvt_d[k])
                    wv_t.append(t)

            def proj_v(mts):
                for mt in mts:
                    ps = pp_pool.tile([P, NQ], f32, tag="pp", name="pp")
                    for k in range(DC):
                        nc.tensor.matmul(
                            ps[:], xt[k][:, mt * P:(mt + 1) * P], wv_t[k][:],
                            start=(k == 0), stop=(k == DC - 1),
                        )
                    for half in range(2):
                        t = v_pool.tile([P, 4, 65], f16, tag=f"v{half}_{mt}",
                                        name=f"v{half}_{mt}")
                        nc.vector.memset(t[:], 1.0)
                        v_half[half][mt] = t
                        src = ps[:, half * 256:half * 256 + 256].rearrange(
                            "p (h d) -> p h d", h=4)
                        nc.vector.tensor_copy(t[:, :, 0:64], src)

            def load_w(w_pool, w_dram, p):
                w_t = []
                for k in range(DC):
                    t = w_pool.tile([P, P], f16, tag="w", name="w")
                    nc.sync.dma_start(t[:], w_dram[p, k])
                    w_t.append(t)
                return w_t

            def proj_nt(out, w_t, nt):
                ps = pp_pool.tile([P, NQ], f32, tag="pp", name="pp")
                for k in range(DC):
                    nc.tensor.matmul(
                        ps[:], w_t[k][:], xt[k][:, nt * NQ:(nt + 1) * NQ],
                        start=(k == 0), stop=(k == DC - 1),
                    )
                nc.vector.tensor_copy(out[:, nt * NQ:(nt + 1) * NQ], ps[:])

            def attention(p, QT, KT, ctx_p, post_qt=None, pre_kt=None):
                for qt in range(QT_TILES):
                    avs = [av_pool.tile([65, NQ], f32, tag="av", name=f"av{j}") for j in range(2)]

                    def av_mms(pt, kt):
                        vh = v_half[p // 2]
                        for j in range(2):
                            nc.tensor.matmul(
                                avs[j][:], vh[kt][:, (2 * p + j) % 4, :],
                                pt[:, j, :],
                                start=(kt == 0), stop=(kt == KT_TILES - 1),
                                skip_group_check=True,
                            )

                    prev = None
                    for kt in range(KT_TILES):
                        if pre_kt is not None and qt == 0:
                            pre_kt(kt)
                        st = st_pool.tile([P, 2, NQ], f32, tag="st", name="st")
                        for j in range(2):
                            h = j * 64
                            nc.tensor.matmul(
                                st[:, j, :],
                                KT[h:h + 64, kt * P:(kt + 1) * P],
                                QT[h:h + 64, qt * NQ:(qt + 1) * NQ],
                                start=True, stop=True,
                            )
                        pt = pt_pool.tile([P, 2, NQ], f16, tag="pt", name="pt")
                        nc.scalar.activation(pt[:], st[:], AF.Exp, scale=0.03125)
                        if prev is not None:
                            av_mms(prev, kt - 1)
                        prev = pt
                    av_mms(prev, KT_TILES - 1)
                    for j in range(2):
                        ctx_slice = ctx_p[j * 64:(j + 1) * 64,
                                          qt * NQ:(qt + 1) * NQ]
                        den = dn_pool.tile([65, NQ], f32r, tag="dn", name="dn")
                        nc.vector.tensor_copy(den[64:65, :], avs[j][64:65, :])
                        # unnormalized ctx out of PSUM (releases the av slot
                        # without waiting on the reciprocal chain)
                        nc.vector.tensor_copy(ctx_slice, avs[j][0:64, :])
                        bc = av_pool.tile([P, NQ], f32, tag="av", name="bc")
                        nc.tensor.matmul(
                            bc[:], ones_r[64:65, :], den[64:65, :],
                            start=True, stop=True,
                        )
                        rec = rc_pool.tile([P, NQ], f32, tag="rc", name="rec")
                        nc.vector.reciprocal_approx_fast(rec[:], bc[:])
                        nc.vector.tensor_mul(
                            out=ctx_slice,
                            in0=ctx_slice,
                            in1=rec[j * 64:(j + 1) * 64, :],
                        )
                    if post_qt is not None:
                        post_qt(qt)

            # pipeline: proj(0), proj(1), att(0), proj(2)+oproj(0), ...
            QTs, KTs, ctxs = {}, {}, {}
            out_sb = []
            wot_t = {}
            for _ot in range(D // P):
                _t = ost_pool.tile([P, S], f32, tag=f"ou{_ot}", name=f"ou{_ot}")
                out_sb.append(_t)

            def proj_pair(p):
                wk_t = load_w(wk_pool, wkt_d, p)
                wq_t = load_w(wq_pool, wqt_d, p)
                KT = kt_pool.tile([P, S], f16, tag="t", name="kt_t")
                QT = qt_pool.tile([P, S], f16, tag="t", name="qt_t")
                for nt in range(QT_TILES):
                    proj_nt(KT, wk_t, nt)
                    proj_nt(QT, wq_t, nt)
                KTs[p], QTs[p] = KT, QT
                ctx_t = ctx_pool.tile([P, S], f16, tag=f"ctx{p}", name=f"ctx{p}")
                ctxs[p] = ctx_t

            def load_wot(p):
                th = wot_pool.tile([P, D], f16, tag=f"woth{p}", name=f"woth{p}")
                nc.sync.dma_start(th[:], woth_d[p])
                wot_t[p] = (th,)

            def oproj_chunk(p, qt, dma_out=False):
                for ot in range(D // P):
                    ps = pp_pool.tile([P, NQ], f32, tag="pp", name="pp")
                    ws = wot_t[p]
                    for i_mm, w in enumerate(ws):
                        nc.tensor.matmul(
                            ps[:], w[:, ot * P:(ot + 1) * P],
                            ctxs[p][:, qt * NQ:(qt + 1) * NQ],
                            start=(i_mm == 0), stop=(i_mm == len(ws) - 1),
                        )
                    dst = out_sb[ot][:, qt * NQ:(qt + 1) * NQ]
                    if p == 0:
                        nc.vector.tensor_copy(dst, ps[:])
                    else:
                        nc.vector.tensor_add(dst, dst, ps[:])
                    if dma_out:
                        nc.sync.dma_start(out_d[ot][:, qt * NQ:(qt + 1) * NQ],
                                          dst)

            def oproj_pair(p):
                for qt in range(QT_TILES):
                    oproj_chunk(p, qt)

            load_wv()
            proj_pair(0)
            attention(0, QTs[0], KTs[0], ctxs[0],
                      pre_kt=lambda kt: proj_v([kt]))
            proj_pair(1)
            attention(1, QTs[1], KTs[1], ctxs[1])
            proj_pair(2)
            load_wot(0)
            oproj_pair(0)
            attention(2, QTs[2], KTs[2], ctxs[2])
            proj_pair(3)
            load_wot(1)
            oproj_pair(1)
            load_wot(2)
            load_wot(3)
            attention(3, QTs[3], KTs[3], ctxs[3],
                      post_qt=lambda qt: oproj_chunk(3, qt))
            oproj_pair(2)
            for _ot in range(D // P):
                nc.sync.dma_start(out_d[_ot][:], out_sb[_ot][:])



    nc.finalize()
    return nc


def _get_nc():
    global _NC_CACHE
    if _NC_CACHE is None:
        _NC_CACHE = _build_nc()
    return _NC_CACHE


def _make_in_maps(hidden_state, w_q, w_k, w_v, w_o):
    hidden_state = np.asarray(hidden_state, np.float32)
    w_q = np.asarray(w_q, np.float32)
    w_k = np.asarray(w_k, np.float32)
    w_v = np.asarray(w_v, np.float32)
    w_o = np.asarray(w_o, np.float32)

    in_maps = []
    for core in range(NCORES):
        b, hh = core // 2, core % 2
        rows = slice(hh * 512, (hh + 1) * 512)
        xt = hidden_state[b].T.astype(np.float16).reshape(DC, P, S)
        # w[rows].T: [1024 d, 512 c] -> (pair, k) chunks [4, 8, 128, 128]
        wqt = (w_q[rows].T.reshape(DC, P, PAIRS, P).transpose(2, 0, 1, 3)
               .astype(np.float16))
        wkt = (w_k[rows].T.reshape(DC, P, PAIRS, P).transpose(2, 0, 1, 3)
               .astype(np.float16))
        wvt = w_v[rows].T.reshape(DC, P, 512).astype(np.float16)
        wot = np.ascontiguousarray(w_o[:, rows].T.reshape(PAIRS, P, D),
                                   np.float32)
        woth = wot.astype(np.float16)
        in_maps.append({"xt": np.ascontiguousarray(xt),
                        "wqt": np.ascontiguousarray(wqt),
                        "wkt": np.ascontiguousarray(wkt),
                        "wvt": np.ascontiguousarray(wvt),
                        "woth": woth})
    return in_maps


def _assemble(results):
    out = np.empty((B, S, D), np.float32)
    for b in range(B):
        t = (results[2 * b]["outt"].reshape(D, S).astype(np.float32)
             + results[2 * b + 1]["outt"].reshape(D, S).astype(np.float32))
        out[b] = t.T
    return out


def run_spmd(hidden_state, w_q, w_k, w_v, w_o, **spmd_kwargs):
    """Run the kernel; returns (output, BassKernelResults)."""
    from concourse.bass_utils import run_bass_kernel_spmd

    nc = _get_nc()
    in_maps = _make_in_maps(hidden_state, w_q, w_k, w_v, w_o)
    res = run_bass_kernel_spmd(nc, in_maps, core_ids=list(range(NCORES)),
                               **spmd_kwargs)
    return _assemble(res.results), res


def kernel(hidden_state, attention_mask=None, w_q=None, w_k=None, w_v=None,
           w_o=None):
    out, _ = run_spmd(hidden_state, w_q, w_k, w_v, w_o)
    return out


# revision 29
# speedup vs baseline: 1.1169x; 1.1169x over previous
"""BertAttention Trainium2 kernel (8 NeuronCores, SPMD).

Sharding: core c handles batch b = c//2 and head-half hh = c%2 (8 of 16 heads).
Each core computes q/k/v projections for its 512 head-dims over its batch's
full sequence, per-head attention (no mask, scale 1/sqrt(1024)), and a partial
o-projection over its 512 context dims.  Host sums the two partials per batch.

Device layout (per core):
  xt   [8,128,2048]  f32r  hidden[b].T, d-major chunks
  wqt  [4,8,128,128] f32r  w_q rows for our heads, transposed, (pair, k) chunks
  wkt  [4,8,128,128] f32r  same for w_k
  wvt  [2,8,128,256] f32r  w_v rows transposed, (col-half, k) chunks
  wot  [4,128,1024]  f32r  w_o cols for our heads, transposed, pair chunks
  outt [8,128,2048]  f32   out_partial.T (o-major chunks)

Attention per head-pair p (heads 2p, 2p+1 local):
  QT/KT [128, 2048] = heads' q/k transposed (head on partitions 0:64 / 64:128)
  S^T tile [128k, 2, 512q]: two row-packed matmuls (K=64 at base 0 and 64)
  exp: one activation over [128, 1024] with fused 1/32 scale -> f32r
  AV: per head, lhsT = V_aug[kt][:, head, 0:65] (64 v-cols + ones col),
      rhs = P^T chunk, accumulated over 16 k-tiles -> psum [65, 512]
      row 64 = softmax denominator.
  norm: den row -> ones-matmul broadcast [64,512] -> DVE recip -> DVE mult
"""

import sys

sys.path.insert(0, "/opt/trn_rl_repo")

import numpy as np

B, S, D, H = 4, 2048, 1024, 16
HEAD = 64
NCORES = 8
P = 128
NQ = 512            # q free-tile width
KT_TILES = S // P   # 16 k tiles
QT_TILES = S // NQ  # 4 q tiles
DC = 8              # contraction chunks for projections (1024/128)
PAIRS = 4           # head pairs per core


def _round_fp32r(x: np.ndarray) -> np.ndarray:
    """RNE-round fp32 to the 11-bit-mantissa fp32r grid (low 12 bits zero)."""
    b = np.ascontiguousarray(x, np.float32).view(np.uint32).astype(np.uint64)
    lsb = (b >> 12) & 1
    b = b + 0x7FF + lsb
    b = (b & 0xFFFFF000) & 0xFFFFFFFF
    return b.astype(np.uint32).view(np.float32)


_NC_CACHE = None


def _build_nc():
    import concourse.bass as bass  # noqa: F401
    import concourse.tile as tile
    from concourse import bacc, mybir

    f32 = mybir.dt.float32
    f32r = mybir.dt.float32r
    f16 = mybir.dt.float16
    AF = mybir.ActivationFunctionType

    nc = bacc.Bacc(None)
    xt_d = nc.declare_dram_parameter("xt", [DC, P, S], f16, isOutput=False)
    wqt_d = nc.declare_dram_parameter("wqt", [PAIRS, DC, P, P], f16, isOutput=False)
    wkt_d = nc.declare_dram_parameter("wkt", [PAIRS, DC, P, P], f16, isOutput=False)
    wvt_d = nc.declare_dram_parameter("wvt", [DC, P, 512], f16, isOutput=False)
    woth_d = nc.declare_dram_parameter("woth", [PAIRS, P, D], f16, isOutput=False)
    out_d = nc.declare_dram_parameter("outt", [D // P, P, S], f32, isOutput=True)

    from contextlib import ExitStack

    with tile.TileContext(nc) as tc, ExitStack() as es:
        def pool(name, bufs, space="SBUF"):
            return es.enter_context(
                tc.tile_pool(name=name, bufs=bufs, space=space))

        xt_pool = pool("xt", 1)
        wq_pool = pool("wq", 8)
        wk_pool = pool("wk", 8)
        wv_pool = pool("wv", 8)
        qt_pool = pool("qt", 2)
        kt_pool = pool("kt", 2)
        v_pool = pool("v", 1)
        pt_pool = pool("pt", 4)
        ctx_pool = pool("ctx", 1)
        wot_pool = pool("wot", 1)
        ost_pool = pool("ost", 1)
        dn_pool = pool("dn", 1)
        rc_pool = pool("rc", 1)
        on_pool = pool("on", 1)
        pp_pool = pool("pp", 2, "PSUM")
        st_pool = pool("st", 2, "PSUM")
        av_pool = pool("av", 2, "PSUM")

        if True:
            # ones row (f32r) for the denominator broadcast matmul
            ones_f = on_pool.tile([P, P], f32, tag="onf", name="onesf")
            nc.vector.memset(ones_f[:], 1.0)
            ones_r = on_pool.tile([P, P], f32r, tag="onr", name="onesr")
            nc.vector.tensor_copy(ones_r[:], ones_f[:])

            # PE warmup during the initial DMA: keeps HAM at 8/8 so the
            # first projection matmuls run at 2.4 GHz
            wup = on_pool.tile([P, NQ], f16, tag="wup", name="wup")
            nc.vector.memset(wup[:], 0.125)
            wups = pp_pool.tile([P, NQ], f32, tag="pp", name="wups")
            for _ in range(30):
                nc.tensor.matmul(wups[:], wup[:, 0:P], wup[:],
                                 start=True, stop=True)

            # load x^T chunks
            xt = []
            for k in range(DC):
                t = xt_pool.tile([P, S], f16, tag=f"xt{k}", name=f"xt{k}")
                eng = nc.sync if k % 2 == 0 else nc.scalar
                eng.dma_start(t[:], xt_d[k])
                xt.append(t)

            # V_aug: separate tiles per head-half (heads 4h..4h+3); ones col
            # per head at offset 65h+64.  One N=512 projection pass fills both.
            v_half = {0: [None] * KT_TILES, 1: [None] * KT_TILES}
            wv_t = []

            def load_wv():
                for k in range(DC):
                    t = wv_pool.tile([P, NQ], f16, tag="wv", name="wv")
                    nc.scalar.dma_start(t[:], wvt_d[k])
                    wv_t.append(t)

            def proj_v(mts):
                for mt in mts:
                    ps = pp_pool.tile([P, NQ], f32, tag="pp", name="pp")
                    for k in range(DC):
                        nc.tensor.matmul(
                            ps[:], xt[k][:, mt * P:(mt + 1) * P], wv_t[k][:],
                            start=(k == 0), stop=(k == DC - 1),
                        )
                    for half in range(2):
                        t = v_pool.tile([P, 4, 65], f16, tag=f"v{half}_{mt}",
                                        name=f"v{half}_{mt}")
                        nc.vector.memset(t[:], 1.0)
                        v_half[half][mt] = t
                        src = ps[:, half * 256:half * 256 + 256].rearrange(
                            "p (h d) -> p h d", h=4)
                        nc.vector.tensor_copy(t[:, :, 0:64], src)

            def load_w(w_pool, w_dram, p):
                w_t = []
                for k in range(DC):
                    t = w_pool.tile([P, P], f16, tag="w", name="w")
                    nc.sync.dma_start(t[:], w_dram[p, k])
                    w_t.append(t)
                return w_t

            def proj_nt(out, w_t, nt):
                ps = pp_pool.tile([P, NQ], f32, tag="pp", name="pp")
                for k in range(DC):
                    nc.tensor.matmul(
                        ps[:], w_t[k][:], xt[k][:, nt * NQ:(nt + 1) * NQ],
                        start=(k == 0), stop=(k == DC - 1),
                    )
                nc.vector.tensor_copy(out[:, nt * NQ:(nt + 1) * NQ], ps[:])

            def attention(p, QT, KT, ctx_p, post_qt=None, pre_kt=None):
                for qt in range(QT_TILES):
                    avs = [av_pool.tile([65, NQ], f32, tag="av", name=f"av{j}") for j in range(2)]

                    def av_mms(pt, kt):
                        vh = v_half[p // 2]
                        for j in range(2):
                            nc.tensor.matmul(
                                avs[j][:], vh[kt][:, (2 * p + j) % 4, :],
                                pt[:, j, :],
                                start=(kt == 0), stop=(kt == KT_TILES - 1),
                                skip_group_check=True,
                            )

                    prev = None
                    for kt in range(KT_TILES):
                        if pre_kt is not None and qt == 0:
                            pre_kt(kt)
                        st = st_pool.tile([P, 2, NQ], f32, tag="st", name="st")
                        for j in range(2):
                            h = j * 64
                            nc.tensor.matmul(
                                st[:, j, :],
                                KT[h:h + 64, kt * P:(kt + 1) * P],
                                QT[h:h + 64, qt * NQ:(qt + 1) * NQ],
                                start=True, stop=True,
                            )
                        pt = pt_pool.tile([P, 2, NQ], f16, tag="pt", name="pt")
                        nc.scalar.activation(pt[:], st[:], AF.Exp, scale=0.03125)
                        if prev is not None:
                            av_mms(prev, kt - 1)
                        prev = pt
                    av_mms(prev, KT_TILES - 1)
                    for j in range(2):
                        ctx_slice = ctx_p[j * 64:(j + 1) * 64,
                                          qt * NQ:(qt + 1) * NQ]
                        den = dn_pool.tile([65, NQ], f32r, tag="dn", name="dn")
                        nc.vector.tensor_copy(den[64:65, :], avs[j][64:65, :])
                        # unnormalized ctx out of PSUM (releases the av slot
                        # without waiting on the reciprocal chain)
                        nc.vector.tensor_copy(ctx_slice, avs[j][0:64, :])
                        bc = av_pool.tile([P, NQ], f32, tag="av", name="bc")
                        nc.tensor.matmul(
                            bc[:], ones_r[64:65, :], den[64:65, :],
                            start=True, stop=True,
                        )
                        rec = rc_pool.tile([P, NQ], f32, tag="rc", name="rec")
                        nc.vector.reciprocal_approx_fast(rec[:], bc[:])
                        nc.vector.tensor_mul(
                            out=ctx_slice,
                            in0=ctx_slice,
                            in1=rec[j * 64:(j + 1) * 64, :],
                        )
                    if post_qt is not None:
                        post_qt(qt)

            # pipeline: proj(0), proj(1), att(0), proj(2)+oproj(0), ...
            QTs, KTs, ctxs = {}, {}, {}
            out_sb = []
            wot_t = {}
            for _ot in range(D // P):
                _t = ost_pool.tile([P, S], f32, tag=f"ou{_ot}", name=f"ou{_ot}")
                out_sb.append(_t)

            def proj_pair(p):
                wk_t = load_w(wk_pool, wkt_d, p)
                wq_t = load_w(wq_pool, wqt_d, p)
                KT = kt_pool.tile([P, S], f16, tag="t", name="kt_t")
                QT = qt_pool.tile([P, S], f16, tag="t", name="qt_t")
                for nt in range(QT_TILES):
                    proj_nt(KT, wk_t, nt)
                    proj_nt(QT, wq_t, nt)
                KTs[p], QTs[p] = KT, QT
                ctx_t = ctx_pool.tile([P, S], f16, tag=f"ctx{p}", name=f"ctx{p}")
                ctxs[p] = ctx_t

            def load_wot(p):
                th = wot_pool.tile([P, D], f16, tag=f"woth{p}", name=f"woth{p}")
                nc.sync.dma_start(th[:], woth_d[p])
                wot_t[p] = (th,)

            def oproj_chunk(p, qt, dma_out=False):
                for ot in range(D // P):
                    ps = pp_pool.tile([P, NQ], f32, tag="pp", name="pp")
                    ws = wot_t[p]
                    for i_mm, w in enumerate(ws):
                        nc.tensor.matmul(
                            ps[:], w[:, ot * P:(ot + 1) * P],
                            ctxs[p][:, qt * NQ:(qt + 1) * NQ],
                            start=(i_mm == 0), stop=(i_mm == len(ws) - 1),
                        )
                    dst = out_sb[ot][:, qt * NQ:(qt + 1) * NQ]
                    if p == 0:
                        nc.vector.tensor_copy(dst, ps[:])
                    else:
                        nc.vector.tensor_add(dst, dst, ps[:])
                    if dma_out:
                        nc.sync.dma_start(out_d[ot][:, qt * NQ:(qt + 1) * NQ],
                                          dst)

            def oproj_pair(p):
                for qt in range(QT_TILES):
                    oproj_chunk(p, qt)

            load_wv()
            proj_pair(0)
            attention(0, QTs[0], KTs[0], ctxs[0],
                      pre_kt=lambda kt: proj_v([kt]))
            proj_pair(1)
            attention(1, QTs[1], KTs[1], ctxs[1])
            proj_pair(2)
            load_wot(0)
            oproj_pair(0)
            attention(2, QTs[2], KTs[2], ctxs[2])
            proj_pair(3)
            load_wot(1)
            oproj_pair(1)
            load_wot(2)
            load_wot(3)
            attention(3, QTs[3], KTs[3], ctxs[3],
                      post_qt=lambda qt: (oproj_chunk(2, qt),
                                          oproj_chunk(3, qt, dma_out=True)))



    nc.finalize()
    return nc


def _get_nc():
    global _NC_CACHE
    if _NC_CACHE is None:
        _NC_CACHE = _build_nc()
    return _NC_CACHE


def _make_in_maps(hidden_state, w_q, w_k, w_v, w_o):
    hidden_state = np.asarray(hidden_state, np.float32)
    w_q = np.asarray(w_q, np.float32)
    w_k = np.asarray(w_k, np.float32)
    w_v = np.asarray(w_v, np.float32)
    w_o = np.asarray(w_o, np.float32)

    in_maps = []
    for core in range(NCORES):
        b, hh = core // 2, core % 2
        rows = slice(hh * 512, (hh + 1) * 512)
        xt = hidden_state[b].T.astype(np.float16).reshape(DC, P, S)
        # w[rows].T: [1024 d, 512 c] -> (pair, k) chunks [4, 8, 128, 128]
        wqt = (w_q[rows].T.reshape(DC, P, PAIRS, P).transpose(2, 0, 1, 3)
               .astype(np.float16))
        wkt = (w_k[rows].T.reshape(DC, P, PAIRS, P).transpose(2, 0, 1, 3)
               .astype(np.float16))
        wvt = w_v[rows].T.reshape(DC, P, 512).astype(np.float16)
        wot = np.ascontiguousarray(w_o[:, rows].T.reshape(PAIRS, P, D),
                                   np.float32)
        woth = wot.astype(np.float16)
        in_maps.append({"xt": np.ascontiguousarray(xt),
                        "wqt": np.ascontiguousarray(wqt),
                        "wkt": np.ascontiguousarray(wkt),
                        "wvt": np.ascontiguousarray(wvt),
                        "woth": woth})
    return in_maps


def _assemble(results):
    out = np.empty((B, S, D), np.float32)
    for b in range(B):
        t = (results[2 * b]["outt"].reshape(D, S).astype(np.float32)
             + results[2 * b + 1]["outt"].reshape(D, S).astype(np.float32))
        out[b] = t.T
    return out


def run_spmd(hidden_state, w_q, w_k, w_v, w_o, **spmd_kwargs):
    """Run the kernel; returns (output, BassKernelResults)."""
    from concourse.bass_utils import run_bass_kernel_spmd

    nc = _get_nc()
    in_maps = _make_in_maps(hidden_state, w_q, w_k, w_v, w_o)
    res = run_bass_kernel_spmd(nc, in_maps, core_ids=list(range(NCORES)),
                               **spmd_kwargs)
    return _assemble(res.results), res


def kernel(hidden_state, attention_mask=None, w_q=None, w_k=None, w_v=None,
           w_o=None):
    out, _ = run_spmd(hidden_state, w_q, w_k, w_v, w_o)
    return out
